# revision 18
# baseline (speedup 1.0000x reference)
"""ECE (expected calibration error) kernel for Trainium2, 8 NeuronCores.

Math: per_bin = |avg_conf - avg_acc| * counts/N  ==  |sum_conf - sum_acc| / N
(when counts>0; both sides 0 when counts==0), so

    ECE = (1/(N*C)) * sum_{b,c} | sum_conf[b,c] - sum_acc[b,c] |

The device computes the heavy O(N*C) part per core (data-parallel over N):
  - V[c]     = sum_n conf[n,c]        (softmax column sums, PE-accumulated)
  - s[n]     = sum_c exp(logits[n,c]) (unshifted; logits bounded, no overflow)
The host assembles the per-(bin,class) sums from these:
  - bin 0 holds every element with conf <= 1/15; V gives its sum_conf column
    totals directly.  Rows whose max confidence can reach 1/15 are recomputed
    exactly on host (a handful of rows) and their >1/15 elements are moved
    from bin 0 into their true bins.
  - sum_acc needs only conf[n, labels[n]] = exp(logits[n,labels[n]]) / s[n].

Device schedule (trace-tuned): the stream is DMA-bound at ~4.8us per 2MB
quad tile (~426 GB/s, the per-core cap), so every other engine must stay
under that rate and the tail chain after the last DMA byte must be short:
  - quad-row tiles [128, 4, 1000]: 16KB contiguous per partition per DMA
    descriptor; one 4000-element ACT exp per tile (3.6us).
  - rowsums via DVE tensor_tensor_reduce on the two 500-col halves of each
    row (bf16 halves add + fp32 accumulate in one op) -- ~2-3us per quad vs
    4.4us for a plain 4000-element tensor_reduce, keeping Vector well under
    the DMA rate.
  - the final two J=1 row blocks fold the rowsum into the Scalar exp via the
    activation accumulator so the after-last-byte chain skips the DVE op.
  - final PSUM->SBUF copies run on Scalar and Vector in parallel.
"""

import os
import sys

import numpy as np

if "/opt/trn_rl_repo" not in sys.path:  # harness may run from a bare dir
    sys.path.insert(0, "/opt/trn_rl_repo")

import concourse.bass as bass
import concourse.tile as tile
from concourse import bacc, mybir
from concourse.bass_utils import run_bass_kernel_spmd

N, C, NB = 65536, 1000, 15
N_CORES = 8
N_LOC = N // N_CORES  # 8192
P = 128
J = 4  # rows per partition per tile
ROWS_PER_TILE = P * J  # 512
T = N_LOC // ROWS_PER_TILE  # 16 tiles per core
NCOL = T * J  # 64 stat columns
H = C // 2  # 500, the half-row width for the fused add+reduce
F32 = mybir.dt.float32
BF16 = mybir.dt.bfloat16

_CACHE: dict = {}
LAST_RESULT = None  # BassKernelResults of the most recent run (for profiling)


def _build():
    nc = bacc.Bacc("TRN2", target_bir_lowering=False, debug=False, num_devices=N_CORES)

    logits_ext = nc.declare_dram_parameter("logits", [N_LOC, C], F32, isOutput=False)
    v_ext = nc.declare_dram_parameter("v_out", [2, C], F32, isOutput=True)
    s_ext = nc.declare_dram_parameter("s_out", [P, NCOL], F32, isOutput=True)

    NA = 500  # first PSUM bank width
    NB_ = C - NA  # second

    with tile.TileContext(nc) as tc:
        with (
            tc.tile_pool(name="xin", bufs=5) as x_pool,
            tc.tile_pool(name="ework", bufs=4) as e_pool,
            tc.tile_pool(name="uhalf", bufs=4) as u_pool,
            tc.tile_pool(name="small", bufs=4) as w_pool,
            tc.tile_pool(name="accum", bufs=1) as acc_pool,
            tc.tile_pool(name="psum", bufs=1, space="PSUM") as psum_pool,
        ):
            s_acc = acc_pool.tile([P, NCOL], F32)
            pA = psum_pool.tile([1, NA], F32)
            pB = psum_pool.tile([1, NB_], F32)
            # Separate PSUM banks for the tail items so the main chain's
            # PSUM->SBUF copies drain while the tail is still computing.
            pC = psum_pool.tile([1, NA], F32)
            pD = psum_pool.tile([1, NB_], F32)
            vout0 = acc_pool.tile([1, C], F32)
            vout1 = acc_pool.tile([1, C], F32)

            # The first block and the last TWO blocks run as J=1 sub-units:
            # the pipeline ramps up after a 512KB DMA instead of a 2MB one,
            # and the last 1MB arrives as 8 small items whose per-item
            # Scalar exp (1.11us) keeps pace with the 1.2us DMA arrival
            # rate, so no compute backlog builds up at stream end (a quad
            # tail leaves Scalar 3.6us and Vector 3.4us of work after the
            # last byte).
            # Work items: (dram_row_start, rows_per_partition, s_col_start)
            work = [(k * P, 1, k) for k in range(J)]
            work += [(t * ROWS_PER_TILE, J, t * J) for t in range(1, T - 2)]
            work += [
                ((T - 2) * ROWS_PER_TILE + k * P, 1, (T - 2) * J + k)
                for k in range(2 * J)
            ]

            n_items = len(work)
            n_tail = 2 * J  # final J=1 blocks, off the quad path
            for it, (row0, jj, col0) in enumerate(work):
                x = x_pool.tile([P, jj, C], F32, tag=f"x{jj}")
                src = logits_ext[row0 : row0 + P * jj, :].rearrange(
                    "(p j) c -> p j c", j=jj
                )
                nc.sync.dma_start(out=x[:], in_=src)

                e = e_pool.tile([P, jj, C], BF16, tag=f"e{jj}")
                if it >= n_items - n_tail:
                    # Tail J=1 blocks accumulate into their own PSUM banks
                    # (pC/pD) so the main chain's stop + copies drain while
                    # the tail streams.  The last two fold the rowsum into
                    # Scalar's activation accumulator (0.28us read vs a
                    # 1.05us DVE add+reduce on the final-byte chain).
                    if it >= n_items - 2:
                        nc.scalar.activation(
                            e[:, 0, :],
                            x[:, 0, :],
                            mybir.ActivationFunctionType.Exp,
                            accum_out=s_acc[:, col0 : col0 + 1],
                        )
                    else:
                        nc.scalar.activation(
                            e[:, 0, :], x[:, 0, :],
                            mybir.ActivationFunctionType.Exp,
                        )
                        nc.vector.tensor_reduce(
                            s_acc[:, col0 : col0 + 1],
                            e[:],
                            axis=mybir.AxisListType.X,
                            op=mybir.AluOpType.add,
                        )
                    wt32 = w_pool.tile([P, 1], F32, tag="wt32")
                    nc.vector.reciprocal(wt32[:], s_acc[:, col0 : col0 + 1])
                    w16 = w_pool.tile([P, 1], BF16, tag="wt")
                    nc.vector.tensor_copy(w16[:], wt32[:])
                    first = it == n_items - n_tail
                    last = it == n_items - 1
                    nc.tensor.matmul(
                        pC[:], w16[:], e[:, 0, :NA], start=first, stop=last
                    )
                    nc.tensor.matmul(
                        pD[:], w16[:], e[:, 0, NA:], start=first, stop=last
                    )
                    if it == n_items - n_tail + 4:
                        # Main chain stopped ~2 items ago; drain its PSUM
                        # banks on Vector mid-tail (emitting earlier would
                        # head-block the queue on the main matmuls' stop).
                        nc.vector.tensor_copy(vout0[:, :NA], pA[:])
                        nc.vector.tensor_copy(vout0[:, NA:], pB[:])
                        nc.sync.dma_start(out=v_ext[0:1, :], in_=vout0[:])
                    continue

                nc.scalar.activation(
                    e[:], x[:], mybir.ActivationFunctionType.Exp
                )
                # Two-stage rowsum: halves-add in bf16 (2x perf mode)
                # then a half-width reduce -- ~3.4us/quad vs 4.4us for a
                # plain 4000-element reduce, keeping DVE under DMA rate.
                u = u_pool.tile([P, jj, H], BF16, tag=f"u{jj}")
                nc.vector.tensor_add(u[:], e[:, :, :H], e[:, :, H:])
                nc.vector.tensor_reduce(
                    s_acc[:, col0 : col0 + jj],
                    u[:],
                    axis=mybir.AxisListType.X,
                    op=mybir.AluOpType.add,
                )

                w32 = w_pool.tile([P, jj], F32, tag=f"w32{jj}")
                nc.vector.reciprocal(w32[:], s_acc[:, col0 : col0 + jj])
                w16 = w_pool.tile([P, jj], BF16, tag=f"w16{jj}")
                nc.vector.tensor_copy(w16[:], w32[:])

                for j in range(jj):
                    first = it == 0 and j == 0
                    last = it == n_items - n_tail - 1 and j == jj - 1
                    nc.tensor.matmul(
                        pA[:], w16[:, j : j + 1], e[:, j, :NA], start=first, stop=last
                    )
                    nc.tensor.matmul(
                        pB[:], w16[:, j : j + 1], e[:, j, NA:], start=first, stop=last
                    )

            nc.sync.dma_start(out=s_ext[:], in_=s_acc[:])
            # Tail-chain PSUM drains, split across Scalar and Vector.
            nc.scalar.copy(vout1[:, :NA], pC[:])
            nc.vector.tensor_copy(vout1[:, NA:], pD[:])
            nc.sync.dma_start(out=v_ext[1:2, :], in_=vout1[:])

    nc.compile()
    return nc


def _get_nc():
    if "nc" not in _CACHE:
        _CACHE["nc"] = _build()
    return _CACHE["nc"]


def _unscramble(a: np.ndarray) -> np.ndarray:
    # Block 0 and blocks T-2, T-1 ran as J=1 units: a[r, t*J + k] holds row
    # t*ROWS_PER_TILE + k*P + r.  Quad blocks 1..T-3: a[r, t*J + j] holds
    # row t*ROWS_PER_TILE + r*J + j.
    out = np.empty(N_LOC, dtype=a.dtype)
    out[:ROWS_PER_TILE] = a[:, :J].T.reshape(ROWS_PER_TILE)
    out[ROWS_PER_TILE : (T - 2) * ROWS_PER_TILE] = (
        a[:, J : (T - 2) * J].reshape(P, T - 3, J).transpose(1, 0, 2).reshape(-1)
    )
    out[(T - 2) * ROWS_PER_TILE :] = a[:, (T - 2) * J :].T.reshape(2 * ROWS_PER_TILE)
    return out


def kernel(logits: np.ndarray, labels: np.ndarray) -> np.ndarray:
    global LAST_RESULT
    logits = np.ascontiguousarray(logits, dtype=np.float32)
    labels_i = np.asarray(labels).astype(np.int64)

    nc = _get_nc()
    in_maps = [
        {"logits": logits[i * N_LOC : (i + 1) * N_LOC]} for i in range(N_CORES)
    ]
    res = run_bass_kernel_spmd(
        nc,
        in_maps,
        core_ids=list(range(N_CORES)),
        trace=os.environ.get("KERNEL_TRACE", "") == "1",
    )
    LAST_RESULT = res
    outs = res.results

    # --- host reassembly (tiny) ---
    V = np.zeros(C, dtype=np.float64)
    s_glob = np.empty(N, dtype=np.float64)
    for i in range(N_CORES):
        V += np.asarray(outs[i]["v_out"]).astype(np.float64).sum(axis=0)
        sl = slice(i * N_LOC, (i + 1) * N_LOC)
        s_glob[sl] = _unscramble(np.asarray(outs[i]["s_out"]).astype(np.float64))

    sumC = np.zeros((NB, C), dtype=np.float64)
    sumA = np.zeros((NB, C), dtype=np.float64)

    # accuracy side: only conf[n, labels[n]] matters
    lg_label = logits[np.arange(N), labels_i].astype(np.float64)
    conf_label = np.exp(lg_label) / s_glob
    valid = conf_label > 0.0
    bl = np.clip(np.ceil(conf_label * NB).astype(np.int64) - 1, 0, NB - 1)
    np.add.at(sumA, (bl[valid], labels_i[valid]), 1.0)

    # confidence side: everything starts in bin 0 via V; move the rare
    # elements with conf > 1/15 into their true bins (exact host recompute).
    # max conf per row = exp(rowmax) / s; rowmax is a cheap host pass.
    maxconf = np.exp(logits.max(axis=1).astype(np.float64)) / s_glob
    flagged = np.nonzero(maxconf > (1.0 / NB) * 0.98)[0]
    if flagged.size:
        xr = logits[flagged].astype(np.float64)
        er = np.exp(xr - xr.max(axis=1, keepdims=True))
        cr = er / er.sum(axis=1, keepdims=True)
        rows, cols = np.nonzero(cr > 1.0 / NB)
        if rows.size:
            vals = cr[rows, cols]
            bins = np.clip(np.ceil(vals * NB).astype(np.int64) - 1, 0, NB - 1)
            np.add.at(sumC, (bins, cols), vals)
            np.subtract.at(V, cols, vals)
    sumC[0] += V

    ece = np.abs(sumC - sumA).sum() / (N * C)
    return np.array([ece], dtype=np.float32)


# revision 19
# speedup vs baseline: 1.0355x; 1.0355x over previous
"""ECE (expected calibration error) kernel for Trainium2, 8 NeuronCores.

Math: per_bin = |avg_conf - avg_acc| * counts/N  ==  |sum_conf - sum_acc| / N
(when counts>0; both sides 0 when counts==0), so

    ECE = (1/(N*C)) * sum_{b,c} | sum_conf[b,c] - sum_acc[b,c] |

The device computes the heavy O(N*C) part per core (data-parallel over N):
  - V[c]     = sum_n conf[n,c]        (softmax column sums, PE-accumulated)
  - s[n]     = sum_c exp(logits[n,c]) (unshifted; logits bounded, no overflow)
  - max_e[n] = max_c exp(logits[n,c]) (so host can flag rows near bin edges)
The host assembles the per-(bin, class) sums from these:
  - bin 0 holds every element with conf <= 1/15; V gives its sum_conf column
    totals directly.  Rows whose max confidence max_e/s can reach 1/15 are
    recomputed exactly on host (a handful of rows) and their >1/15 elements
    are moved from bin 0 into their true bins.
  - sum_acc needs only conf[n, labels[n]] = exp(logits[n,labels[n]]) / s[n].

Device layout: quad-row tiles [128, 4, 1000] where partition r holds DRAM
rows 4r..4r+3 of the 512-row block -> 16KB contiguous per partition per DMA
descriptor; one 4000-element ACT exp op per tile; bf16 4x-mode Vector
reductions; per-row 1/s folded into the PE column-sum as the stationary.
"""

import os
import sys

import numpy as np

if "/opt/trn_rl_repo" not in sys.path:  # harness may run from a bare dir
    sys.path.insert(0, "/opt/trn_rl_repo")

import concourse.bass as bass
import concourse.tile as tile
from concourse import bacc, mybir
from concourse.bass_utils import run_bass_kernel_spmd

N, C, NB = 65536, 1000, 15
N_CORES = 8
N_LOC = N // N_CORES  # 8192
P = 128
J = 4  # rows per partition per tile
ROWS_PER_TILE = P * J  # 512
T = N_LOC // ROWS_PER_TILE  # 16 tiles per core
NCOL = T * J  # 64 stat columns
F32 = mybir.dt.float32
BF16 = mybir.dt.bfloat16

_CACHE: dict = {}
LAST_RESULT = None  # BassKernelResults of the most recent run (for profiling)


def _build():
    nc = bacc.Bacc("TRN2", target_bir_lowering=False, debug=False, num_devices=N_CORES)

    logits_ext = nc.declare_dram_parameter("logits", [N_LOC, C], F32, isOutput=False)
    v_ext = nc.declare_dram_parameter("v_out", [1, C], F32, isOutput=True)
    s_ext = nc.declare_dram_parameter("s_out", [P, NCOL], F32, isOutput=True)

    NA = 500  # first PSUM bank width
    NB_ = C - NA  # second

    with tile.TileContext(nc) as tc:
        with (
            tc.tile_pool(name="xin", bufs=5) as x_pool,
            tc.tile_pool(name="ework", bufs=4) as e_pool,
            tc.tile_pool(name="small", bufs=4) as w_pool,
            tc.tile_pool(name="accum", bufs=1) as acc_pool,
            tc.tile_pool(name="psum", bufs=1, space="PSUM") as psum_pool,
        ):
            s_acc = acc_pool.tile([P, NCOL], F32)
            pA = psum_pool.tile([1, NA], F32)
            pB = psum_pool.tile([1, NB_], F32)

            # The first and last 512-row blocks run as four J=1 sub-units:
            # the pipeline ramps up after a 512KB DMA instead of a 2MB one,
            # and the tail chain after the final DMA holds one 1.5us
            # exp+accum instead of four.
            # Work items: (dram_row_start, rows_per_partition, s_col_start)
            work = [(k * P, 1, k) for k in range(J)]
            work += [(t * ROWS_PER_TILE, J, t * J) for t in range(1, T - 1)]
            work += [
                ((T - 1) * ROWS_PER_TILE + k * P, 1, (T - 1) * J + k)
                for k in range(J)
            ]

            n_items = len(work)
            for it, (row0, jj, col0) in enumerate(work):
                x = x_pool.tile([P, jj, C], F32, tag=f"x{jj}")
                src = logits_ext[row0 : row0 + P * jj, :].rearrange(
                    "(p j) c -> p j c", j=jj
                )
                nc.sync.dma_start(out=x[:], in_=src)

                e = e_pool.tile([P, jj, C], BF16, tag=f"e{jj}")
                if it >= n_items - 6:
                    # Last tiles: rowsum via Scalar's activation accumulator
                    # (per-row exp ops) so Vector drains early and the tail
                    # chain skips its 4.3us reduce.  Balances Scalar ~64us
                    # vs Vector ~61us, both well under DMA's ~78us.
                    for j in range(jj):
                        nc.scalar.activation(
                            e[:, j, :],
                            x[:, j, :],
                            mybir.ActivationFunctionType.Exp,
                            accum_out=s_acc[:, col0 + j : col0 + j + 1],
                        )
                else:
                    nc.scalar.activation(
                        e[:], x[:], mybir.ActivationFunctionType.Exp
                    )
                    nc.vector.tensor_reduce(
                        s_acc[:, col0 : col0 + jj],
                        e[:],
                        axis=mybir.AxisListType.X,
                        op=mybir.AluOpType.add,
                    )

                w32 = w_pool.tile([P, jj], F32, tag=f"w32{jj}")
                nc.vector.reciprocal(w32[:], s_acc[:, col0 : col0 + jj])
                w16 = w_pool.tile([P, jj], BF16, tag=f"w16{jj}")
                nc.vector.tensor_copy(w16[:], w32[:])

                for j in range(jj):
                    first = it == 0 and j == 0
                    last = it == n_items - 1 and j == jj - 1
                    nc.tensor.matmul(
                        pA[:], w16[:, j : j + 1], e[:, j, :NA], start=first, stop=last
                    )
                    nc.tensor.matmul(
                        pB[:], w16[:, j : j + 1], e[:, j, NA:], start=first, stop=last
                    )

            vout = acc_pool.tile([1, C], F32)
            nc.scalar.copy(vout[:, :NA], pA[:])
            nc.scalar.copy(vout[:, NA:], pB[:])
            nc.sync.dma_start(out=v_ext[:], in_=vout[:])
            nc.sync.dma_start(out=s_ext[:], in_=s_acc[:])

    nc.compile()
    return nc


def _get_nc():
    if "nc" not in _CACHE:
        _CACHE["nc"] = _build()
    return _CACHE["nc"]


def _unscramble(a: np.ndarray) -> np.ndarray:
    # Blocks 0 and T-1 ran as four J=1 units: a[r, t*J + k] holds row
    # t*ROWS_PER_TILE + k*P + r.  Quad blocks 1..T-2: a[r, t*J + j] holds
    # row t*ROWS_PER_TILE + r*J + j.
    out = np.empty(N_LOC, dtype=a.dtype)
    out[:ROWS_PER_TILE] = a[:, :J].T.reshape(ROWS_PER_TILE)
    out[ROWS_PER_TILE : (T - 1) * ROWS_PER_TILE] = (
        a[:, J : (T - 1) * J].reshape(P, T - 2, J).transpose(1, 0, 2).reshape(-1)
    )
    out[(T - 1) * ROWS_PER_TILE :] = a[:, (T - 1) * J :].T.reshape(ROWS_PER_TILE)
    return out


def kernel(logits: np.ndarray, labels: np.ndarray) -> np.ndarray:
    global LAST_RESULT
    logits = np.ascontiguousarray(logits, dtype=np.float32)
    labels_i = np.asarray(labels).astype(np.int64)

    nc = _get_nc()
    in_maps = [
        {"logits": logits[i * N_LOC : (i + 1) * N_LOC]} for i in range(N_CORES)
    ]
    res = run_bass_kernel_spmd(
        nc,
        in_maps,
        core_ids=list(range(N_CORES)),
        trace=os.environ.get("KERNEL_TRACE", "") == "1",
    )
    LAST_RESULT = res
    outs = res.results

    # --- host reassembly (tiny) ---
    V = np.zeros(C, dtype=np.float64)
    s_glob = np.empty(N, dtype=np.float64)
    for i in range(N_CORES):
        V += np.asarray(outs[i]["v_out"]).reshape(C).astype(np.float64)
        sl = slice(i * N_LOC, (i + 1) * N_LOC)
        s_glob[sl] = _unscramble(np.asarray(outs[i]["s_out"]).astype(np.float64))

    sumC = np.zeros((NB, C), dtype=np.float64)
    sumA = np.zeros((NB, C), dtype=np.float64)

    # accuracy side: only conf[n, labels[n]] matters
    lg_label = logits[np.arange(N), labels_i].astype(np.float64)
    conf_label = np.exp(lg_label) / s_glob
    valid = conf_label > 0.0
    bl = np.clip(np.ceil(conf_label * NB).astype(np.int64) - 1, 0, NB - 1)
    np.add.at(sumA, (bl[valid], labels_i[valid]), 1.0)

    # confidence side: everything starts in bin 0 via V; move the rare
    # elements with conf > 1/15 into their true bins (exact host recompute).
    # max conf per row = exp(rowmax) / s; rowmax is a cheap host pass.
    maxconf = np.exp(logits.max(axis=1).astype(np.float64)) / s_glob
    flagged = np.nonzero(maxconf > (1.0 / NB) * 0.98)[0]
    if flagged.size:
        xr = logits[flagged].astype(np.float64)
        er = np.exp(xr - xr.max(axis=1, keepdims=True))
        cr = er / er.sum(axis=1, keepdims=True)
        rows, cols = np.nonzero(cr > 1.0 / NB)
        if rows.size:
            vals = cr[rows, cols]
            bins = np.clip(np.ceil(vals * NB).astype(np.int64) - 1, 0, NB - 1)
            np.add.at(sumC, (bins, cols), vals)
            np.subtract.at(V, cols, vals)
    sumC[0] += V

    ece = np.abs(sumC - sumA).sum() / (N * C)
    return np.array([ece], dtype=np.float32)


# revision 20
# speedup vs baseline: 1.0372x; 1.0017x over previous
"""ECE (expected calibration error) kernel for Trainium2, 8 NeuronCores.

Math: per_bin = |avg_conf - avg_acc| * counts/N  ==  |sum_conf - sum_acc| / N
(when counts>0; both sides 0 when counts==0), so

    ECE = (1/(N*C)) * sum_{b,c} | sum_conf[b,c] - sum_acc[b,c] |

The device computes the heavy O(N*C) part per core (data-parallel over N):
  - V[c]     = sum_n conf[n,c]        (softmax column sums, PE-accumulated)
  - s[n]     = sum_c exp(logits[n,c]) (unshifted; logits bounded, no overflow)
  - max_e[n] = max_c exp(logits[n,c]) (so host can flag rows near bin edges)
The host assembles the per-(bin, class) sums from these:
  - bin 0 holds every element with conf <= 1/15; V gives its sum_conf column
    totals directly.  Rows whose max confidence max_e/s can reach 1/15 are
    recomputed exactly on host (a handful of rows) and their >1/15 elements
    are moved from bin 0 into their true bins.
  - sum_acc needs only conf[n, labels[n]] = exp(logits[n,labels[n]]) / s[n].

Device layout: quad-row tiles [128, 4, 1000] where partition r holds DRAM
rows 4r..4r+3 of the 512-row block -> 16KB contiguous per partition per DMA
descriptor; one 4000-element ACT exp op per tile; bf16 4x-mode Vector
reductions; per-row 1/s folded into the PE column-sum as the stationary.
"""

import os
import sys

import numpy as np

if "/opt/trn_rl_repo" not in sys.path:  # harness may run from a bare dir
    sys.path.insert(0, "/opt/trn_rl_repo")

import concourse.bass as bass
import concourse.tile as tile
from concourse import bacc, mybir
from concourse.bass_utils import run_bass_kernel_spmd

N, C, NB = 65536, 1000, 15
N_CORES = 8
N_LOC = N // N_CORES  # 8192
P = 128
J = 4  # rows per partition per tile
ROWS_PER_TILE = P * J  # 512
T = N_LOC // ROWS_PER_TILE  # 16 tiles per core
NCOL = T * J  # 64 stat columns
F32 = mybir.dt.float32
BF16 = mybir.dt.bfloat16

_CACHE: dict = {}
LAST_RESULT = None  # BassKernelResults of the most recent run (for profiling)


def _build():
    nc = bacc.Bacc("TRN2", target_bir_lowering=False, debug=False, num_devices=N_CORES)

    logits_ext = nc.declare_dram_parameter("logits", [N_LOC, C], F32, isOutput=False)
    v_ext = nc.declare_dram_parameter("v_out", [1, C], F32, isOutput=True)
    s_ext = nc.declare_dram_parameter("s_out", [P, NCOL], F32, isOutput=True)

    NA = 500  # first PSUM bank width
    NB_ = C - NA  # second

    with tile.TileContext(nc) as tc:
        with (
            tc.tile_pool(name="xin", bufs=6) as x_pool,
            tc.tile_pool(name="ework", bufs=5) as e_pool,
            tc.tile_pool(name="small", bufs=4) as w_pool,
            tc.tile_pool(name="accum", bufs=1) as acc_pool,
            tc.tile_pool(name="psum", bufs=1, space="PSUM") as psum_pool,
        ):
            s_acc = acc_pool.tile([P, NCOL], F32)
            pA = psum_pool.tile([1, NA], F32)
            pB = psum_pool.tile([1, NB_], F32)

            # The first and last 512-row blocks run as four J=1 sub-units:
            # the pipeline ramps up after a 512KB DMA instead of a 2MB one,
            # and the tail chain after the final DMA holds one 1.5us
            # exp+accum instead of four.
            # Work items: (dram_row_start, rows_per_partition, s_col_start)
            work = [(k * P, 1, k) for k in range(J)]
            work += [(t * ROWS_PER_TILE, J, t * J) for t in range(1, T - 1)]
            work += [
                ((T - 1) * ROWS_PER_TILE + k * P, 1, (T - 1) * J + k)
                for k in range(J)
            ]

            n_items = len(work)
            for it, (row0, jj, col0) in enumerate(work):
                x = x_pool.tile([P, jj, C], F32, tag=f"x{jj}")
                src = logits_ext[row0 : row0 + P * jj, :].rearrange(
                    "(p j) c -> p j c", j=jj
                )
                nc.sync.dma_start(out=x[:], in_=src)

                e = e_pool.tile([P, jj, C], BF16, tag=f"e{jj}")
                if it >= n_items - 6:
                    # Last tiles: rowsum via Scalar's activation accumulator
                    # (per-row exp ops) so Vector drains early and the tail
                    # chain skips its 4.3us reduce.  Balances Scalar ~64us
                    # vs Vector ~61us, both well under DMA's ~78us.
                    for j in range(jj):
                        nc.scalar.activation(
                            e[:, j, :],
                            x[:, j, :],
                            mybir.ActivationFunctionType.Exp,
                            accum_out=s_acc[:, col0 + j : col0 + j + 1],
                        )
                else:
                    nc.scalar.activation(
                        e[:], x[:], mybir.ActivationFunctionType.Exp
                    )
                    nc.vector.tensor_reduce(
                        s_acc[:, col0 : col0 + jj],
                        e[:],
                        axis=mybir.AxisListType.X,
                        op=mybir.AluOpType.add,
                    )

                w32 = w_pool.tile([P, jj], F32, tag=f"w32{jj}")
                nc.vector.reciprocal(w32[:], s_acc[:, col0 : col0 + jj])
                w16 = w_pool.tile([P, jj], BF16, tag=f"w16{jj}")
                nc.vector.tensor_copy(w16[:], w32[:])

                for j in range(jj):
                    first = it == 0 and j == 0
                    last = it == n_items - 1 and j == jj - 1
                    nc.tensor.matmul(
                        pA[:], w16[:, j : j + 1], e[:, j, :NA], start=first, stop=last
                    )
                    nc.tensor.matmul(
                        pB[:], w16[:, j : j + 1], e[:, j, NA:], start=first, stop=last
                    )

            vout = acc_pool.tile([1, C], F32)
            nc.scalar.copy(vout[:, :NA], pA[:])
            nc.scalar.copy(vout[:, NA:], pB[:])
            nc.sync.dma_start(out=v_ext[:], in_=vout[:])
            nc.sync.dma_start(out=s_ext[:], in_=s_acc[:])

    nc.compile()
    return nc


def _get_nc():
    if "nc" not in _CACHE:
        _CACHE["nc"] = _build()
    return _CACHE["nc"]


def _unscramble(a: np.ndarray) -> np.ndarray:
    # Blocks 0 and T-1 ran as four J=1 units: a[r, t*J + k] holds row
    # t*ROWS_PER_TILE + k*P + r.  Quad blocks 1..T-2: a[r, t*J + j] holds
    # row t*ROWS_PER_TILE + r*J + j.
    out = np.empty(N_LOC, dtype=a.dtype)
    out[:ROWS_PER_TILE] = a[:, :J].T.reshape(ROWS_PER_TILE)
    out[ROWS_PER_TILE : (T - 1) * ROWS_PER_TILE] = (
        a[:, J : (T - 1) * J].reshape(P, T - 2, J).transpose(1, 0, 2).reshape(-1)
    )
    out[(T - 1) * ROWS_PER_TILE :] = a[:, (T - 1) * J :].T.reshape(ROWS_PER_TILE)
    return out


def kernel(logits: np.ndarray, labels: np.ndarray) -> np.ndarray:
    global LAST_RESULT
    logits = np.ascontiguousarray(logits, dtype=np.float32)
    labels_i = np.asarray(labels).astype(np.int64)

    nc = _get_nc()
    in_maps = [
        {"logits": logits[i * N_LOC : (i + 1) * N_LOC]} for i in range(N_CORES)
    ]
    res = run_bass_kernel_spmd(
        nc,
        in_maps,
        core_ids=list(range(N_CORES)),
        trace=os.environ.get("KERNEL_TRACE", "") == "1",
    )
    LAST_RESULT = res
    outs = res.results

    # --- host reassembly (tiny) ---
    V = np.zeros(C, dtype=np.float64)
    s_glob = np.empty(N, dtype=np.float64)
    for i in range(N_CORES):
        V += np.asarray(outs[i]["v_out"]).reshape(C).astype(np.float64)
        sl = slice(i * N_LOC, (i + 1) * N_LOC)
        s_glob[sl] = _unscramble(np.asarray(outs[i]["s_out"]).astype(np.float64))

    sumC = np.zeros((NB, C), dtype=np.float64)
    sumA = np.zeros((NB, C), dtype=np.float64)

    # accuracy side: only conf[n, labels[n]] matters
    lg_label = logits[np.arange(N), labels_i].astype(np.float64)
    conf_label = np.exp(lg_label) / s_glob
    valid = conf_label > 0.0
    bl = np.clip(np.ceil(conf_label * NB).astype(np.int64) - 1, 0, NB - 1)
    np.add.at(sumA, (bl[valid], labels_i[valid]), 1.0)

    # confidence side: everything starts in bin 0 via V; move the rare
    # elements with conf > 1/15 into their true bins (exact host recompute).
    # max conf per row = exp(rowmax) / s; rowmax is a cheap host pass.
    maxconf = np.exp(logits.max(axis=1).astype(np.float64)) / s_glob
    flagged = np.nonzero(maxconf > (1.0 / NB) * 0.98)[0]
    if flagged.size:
        xr = logits[flagged].astype(np.float64)
        er = np.exp(xr - xr.max(axis=1, keepdims=True))
        cr = er / er.sum(axis=1, keepdims=True)
        rows, cols = np.nonzero(cr > 1.0 / NB)
        if rows.size:
            vals = cr[rows, cols]
            bins = np.clip(np.ceil(vals * NB).astype(np.int64) - 1, 0, NB - 1)
            np.add.at(sumC, (bins, cols), vals)
            np.subtract.at(V, cols, vals)
    sumC[0] += V

    ece = np.abs(sumC - sumA).sum() / (N * C)
    return np.array([ece], dtype=np.float32)
